# revision 10
# baseline (speedup 1.0000x reference)
"""Custom cross-entropy-with-top-k loss kernel for Trainium2 (8 NeuronCores).

Reference computation (B=16384 rows, C=8192 classes, K=5, POWER=1.01):
    log_prob      = log_softmax(input)
    topk_vals     = top-5 values per row
    log_prob_topk = log(1.01^topk_vals / sum(1.01^topk_vals))
    log_prob_copy = log_prob with topk positions overwritten by log_prob_topk
    loss = mean(-log_prob[r, target[r]]) + mean(-log_prob_copy[r, target[r]])

Key reduction: the scalar loss needs only, per row,
    lse   = log(sum(exp(x)))               (x ~ N(0,1): exp() safe in f32)
    x_t   = x[row, target[row]]            (indirect-DMA gather)
    top5  = 5 largest values               (VectorE InstMax = top-8)
    sel   = x_t >= top5[4]                 (is target among the top-5)
    lp2   = sel ? ln(1.01)*x_t - log(sum(1.01^top5)) : x_t - lse
    term  = (lse - x_t) - lp2
and the answer is mean(term).  Per core: 2048 rows = 16 tiles of 128
partitions x 8192 f32, streamed at the HBM roofline.

Pipeline structure (v2):
  - Tiles 1..14 load as full 4 MiB HWDGE transfers into a 5-slot SBUF
    rotation; tiles 0 and 15 load as 4x 2048-column chunks.  Chunking
    tile 0 lets ScalarE/VectorE start ~12 us earlier (the per-tile
    MAX8 total of ~167 us/core is near the stream duration, so start
    latency is on the critical path); chunking tile 15 cuts the tail
    after the last HBM byte from ~19 us to ~6 us.
  - ScalarE: one Exp pass per tile/chunk with a per-row accumulator
    (sumexp).  The elementwise output goes to a write-only fp16 sink
    that is never read, so no WAW synchronization is needed on it.
  - VectorE: InstMax (top-8) per tile/chunk; chunked tiles merge via a
    second InstMax over the 4x8 concatenated chunk results (top-5 of a
    row is contained in the union of per-chunk top-8s).
  - GpSimd: one indirect-DMA gather of x[row, target[row]] (drains in
    the first ~15 us of the stream, off the critical path).
  - Epilogue: ln/exp on [128,16]-shaped tiles plus a short DVE chain;
    the final scalar_tensor_tensor emits the per-partition row sum via
    accum_out, fusing the last reduction.

Written in raw Bass (no Tile scheduler): the neuronxcc walrus backend
only encodes ONE semaphore wait per TPB instruction, so synchronization
uses explicit standalone wait_ge instructions (one wait each) and
relies on transitive ordering.
"""

import numpy as np

P = 128                    # SBUF partitions
C = 8192                   # classes
NTILES = 16                # row-tiles per core
B_LOCAL = P * NTILES       # 2048 rows per core
N_CORES = 8
B = B_LOCAL * N_CORES      # 16384
LN101 = float(np.log(np.float64(1.01)))

NB = 5                     # x-tile rotation depth
NCH = 4                    # column chunks for the first/last tile
CW = C // NCH              # 2048 columns per chunk (last tile, uniform)
# First tile uses uneven chunks so the first Exp/MAX8 can start as soon
# as possible (the DVE MAX8 total is close to the stream duration, so
# compute start latency is on the critical path).
CH0 = [(0, 1024), (1024, 1024), (2048, 2048), (4096, 4096)]
LN5 = float(np.log(np.float64(5.0)))
_CACHE = {}


def _build_bass():
    from contextlib import ExitStack

    import concourse.bass as bass
    import concourse.mybir as mybir

    nc = bass.Bass()
    f32 = mybir.dt.float32
    f16 = mybir.dt.float16
    x = nc.declare_dram_parameter("x", [B_LOCAL, C], f32, isOutput=False)
    gidx = nc.declare_dram_parameter(
        "gidx", [P, NTILES], mybir.dt.int32, isOutput=False
    )
    out = nc.declare_dram_parameter("out", [P, 1], f32, isOutput=True)

    Exp = mybir.ActivationFunctionType.Exp
    Ln = mybir.ActivationFunctionType.Ln
    X = mybir.AxisListType.X
    Alu = mybir.AluOpType

    # s_act increments (all on ScalarE, in program order):
    #   tile0 chunks: 1..4 ; tiles 1..14: 4+i ; tile15 chunks: 19..22 ;
    #   lse: 23
    def act_done(j):       # s_act value once tile j's exp(+accum) is done
        return 4 if j == 0 else 4 + j

    # s_dve increments (all on VectorE, in program order):
    #   tile0 chunk maxes: 1..4 ; merge0: 5 ; se0 reduce: 6 ;
    #   tiles 1..14 maxes: 6+i ; tile15 chunk maxes: 21..24 ; merge15: 25 ;
    #   se15 reduce: 26 ; sred: 27 ; logs: 28 ; a: 29 ; sel: 30 ; u: 31 ;
    #   d: 32 ; m: 33 ; term(+accum): 34
    def dve_done(j):       # s_dve value once tile j's slot reads are done
        return 4 if j == 0 else 6 + j

    DVE_FINAL = 34

    with ExitStack() as ctx:
        xt = [
            ctx.enter_context(nc.sbuf_tensor(f"xt{j}", [P, C], f32))
            for j in range(NB)
        ]
        # Write-only elementwise sink for the Exp passes (never read).
        scr = ctx.enter_context(nc.sbuf_tensor("scr", [P, C], f16))
        gidx_sb = ctx.enter_context(
            nc.sbuf_tensor("gidx_sb", [P, NTILES], mybir.dt.int32)
        )
        xt_all = ctx.enter_context(nc.sbuf_tensor("xt_all", [P, NTILES], f32))
        top8_all = ctx.enter_context(
            nc.sbuf_tensor("top8_all", [P, NTILES, 8], f32)
        )
        top8c0 = ctx.enter_context(nc.sbuf_tensor("top8c0", [P, NCH, 8], f32))
        top8c15 = ctx.enter_context(nc.sbuf_tensor("top8c15", [P, NCH, 8], f32))
        se_all = ctx.enter_context(nc.sbuf_tensor("se_all", [P, NTILES], f32))
        sec0 = ctx.enter_context(nc.sbuf_tensor("sec0", [P, NCH], f32))
        sec15 = ctx.enter_context(nc.sbuf_tensor("sec15", [P, NCH], f32))
        lse_all = ctx.enter_context(nc.sbuf_tensor("lse_all", [P, NTILES], f32))
        s_red = ctx.enter_context(nc.sbuf_tensor("s_red", [P, NTILES], f32))
        logs_all = ctx.enter_context(
            nc.sbuf_tensor("logs_all", [P, NTILES], f32)
        )
        a_all = ctx.enter_context(nc.sbuf_tensor("a_all", [P, NTILES], f32))
        u_all = ctx.enter_context(nc.sbuf_tensor("u_all", [P, NTILES], f32))
        d_all = ctx.enter_context(nc.sbuf_tensor("d_all", [P, NTILES], f32))
        sel_all = ctx.enter_context(nc.sbuf_tensor("sel_all", [P, NTILES], f32))
        term_all = ctx.enter_context(
            nc.sbuf_tensor("term_all", [P, NTILES], f32)
        )
        partial = ctx.enter_context(nc.sbuf_tensor("partial", [P, 1], f32))

        s_gidx = ctx.enter_context(nc.semaphore("s_gidx"))
        # One semaphore per tile load: a semaphore's first increment needs
        # no prior wait by the enqueuing engine, so the DMA queue can run
        # arbitrarily far ahead.  Chunked tiles increment theirs by 16 per
        # chunk (consumers wait for 16*(c+1)).
        s_load = [
            ctx.enter_context(nc.semaphore(f"s_load{i}")) for i in range(NTILES)
        ]
        s_store = ctx.enter_context(nc.semaphore("s_store"))
        s_gather = ctx.enter_context(nc.semaphore("s_gather"))
        s_act = ctx.enter_context(nc.semaphore("s_act"))
        s_dve = ctx.enter_context(nc.semaphore("s_dve"))
        block = ctx.enter_context(nc.Block())

        def cs(c):
            return slice(c * CW, (c + 1) * CW)

        @block.sync
        def _(sync):
            # tile 0 as 4 uneven column chunks (compute starts on the small
            # first chunk ~15 us before a full-tile load would complete)
            for off, w in CH0:
                sync.dma_start(
                    out=xt[0][:, off : off + w], in_=x[0:P, off : off + w]
                ).then_inc(s_load[0], 16)
            # gidx after the tile-0 chunks: the gather result is only
            # needed in the epilogue.
            sync.dma_start(out=gidx_sb[:, :], in_=gidx[:, :]).then_inc(s_gidx, 16)
            # full tiles 1..14
            for i in range(1, NTILES - 1):
                if i >= NB:
                    # Slot reuse: wait for both compute consumers of the
                    # previous occupant (tile i-NB).  Their completion also
                    # implies that load's completion, covering the slot WAW
                    # transitively.
                    sync.wait_ge(s_act, act_done(i - NB))
                    sync.wait_ge(s_dve, dve_done(i - NB))
                sync.dma_start(
                    out=xt[i % NB][:, :], in_=x[i * P : (i + 1) * P, :]
                ).then_inc(s_load[i], 16)
            # tile 15 as 4 column chunks (short tail after the last byte)
            sync.wait_ge(s_act, act_done(NTILES - 1 - NB))
            sync.wait_ge(s_dve, dve_done(NTILES - 1 - NB))
            for c in range(NCH):
                sync.dma_start(
                    out=xt[(NTILES - 1) % NB][:, cs(c)],
                    in_=x[(NTILES - 1) * P : NTILES * P, cs(c)],
                ).then_inc(s_load[NTILES - 1], 16)
            # final store after the whole epilogue
            sync.wait_ge(s_dve, DVE_FINAL)
            sync.dma_start(out=out[:, :], in_=partial[:, :]).then_inc(s_store, 16)

        @block.gpsimd
        def _(gpsimd):
            gpsimd.wait_ge(s_gidx, 16)
            x_flat = bass.AP(tensor=x, offset=0, ap=[[1, B_LOCAL * C], [1, 1]])
            gpsimd.indirect_dma_start(
                out=xt_all[:, :],
                out_offset=None,
                in_=x_flat,
                in_offset=bass.IndirectOffsetOnAxis(ap=gidx_sb[:, :], axis=0),
            ).then_inc(s_gather, 16)

        @block.scalar
        def _(scalar):
            # tile 0 chunks -> s_act 1..4
            for c, (off, w) in enumerate(CH0):
                scalar.wait_ge(s_load[0], 16 * (c + 1))
                scalar.activation(
                    out=scr[:, off : off + w],
                    in_=xt[0][:, off : off + w],
                    func=Exp,
                    accum_out=sec0[:, c : c + 1],
                ).then_inc(s_act, 1)
            # tiles 1..14 -> s_act 4+i
            for i in range(1, NTILES - 1):
                scalar.wait_ge(s_load[i], 16)
                scalar.activation(
                    out=scr[:, :],
                    in_=xt[i % NB][:, :],
                    func=Exp,
                    accum_out=se_all[:, i : i + 1],
                ).then_inc(s_act, 1)
            # tile 15 chunks -> s_act 19..22
            for c in range(NCH):
                scalar.wait_ge(s_load[NTILES - 1], 16 * (c + 1))
                scalar.activation(
                    out=scr[:, cs(c)],
                    in_=xt[(NTILES - 1) % NB][:, cs(c)],
                    func=Exp,
                    accum_out=sec15[:, c : c + 1],
                ).then_inc(s_act, 1)
            # epilogue: lse needs se_all[:,0] and se_all[:,15], written by
            # DVE reduces (s_dve 6 and 26); 26 also implies this engine's
            # own accumulator reads have completed (the DVE reduce waited
            # on s_act>=22).
            scalar.wait_ge(s_dve, 26)
            scalar.activation(
                out=lse_all[:, :], in_=se_all[:, :], func=Ln
            ).then_inc(s_act, 1)  # -> 23

        @block.vector
        def _(vector):
            # tile 0 chunk maxes -> s_dve 1..4
            for c, (off, w) in enumerate(CH0):
                vector.wait_ge(s_load[0], 16 * (c + 1))
                vector.max(
                    out=top8c0[:, c, :], in_=xt[0][:, off : off + w]
                ).then_inc(s_dve, 1)
            # merge0 reads top8c0 (same-engine RAW: self-wait)
            vector.wait_ge(s_dve, 4)
            vector.max(
                out=top8_all[:, 0, :], in_=top8c0[:, :, :]
            ).then_inc(s_dve, 1)  # -> 5
            vector.wait_ge(s_act, 4)  # tile0 chunk accums done
            vector.reduce_sum(
                out=se_all[:, 0:1], in_=sec0[:, :], axis=X
            ).then_inc(s_dve, 1)  # -> 6
            # tiles 1..14 -> s_dve 6+i
            for i in range(1, NTILES - 1):
                vector.wait_ge(s_load[i], 16)
                vector.max(
                    out=top8_all[:, i, :], in_=xt[i % NB][:, :]
                ).then_inc(s_dve, 1)
            # tile 15 chunk maxes -> s_dve 21..24
            for c in range(NCH):
                vector.wait_ge(s_load[NTILES - 1], 16 * (c + 1))
                vector.max(
                    out=top8c15[:, c, :], in_=xt[(NTILES - 1) % NB][:, cs(c)]
                ).then_inc(s_dve, 1)
            vector.wait_ge(s_dve, 24)
            vector.max(
                out=top8_all[:, NTILES - 1, :], in_=top8c15[:, :, :]
            ).then_inc(s_dve, 1)  # -> 25
            vector.wait_ge(s_act, 22)  # tile15 chunk accums done
            vector.reduce_sum(
                out=se_all[:, NTILES - 1 : NTILES], in_=sec15[:, :], axis=X
            ).then_inc(s_dve, 1)  # -> 26
            # epilogue.  s_red = sum of the top-5 values per tile; logs
            # uses log(sum(1.01^v over top5)) = ln5 + ln(1.01)*mean(top5)
            # to second order in ln(1.01)*v (|error| <= ~2e-4, and the
            # term it feeds only applies to rows whose target is in the
            # top-5: 5/8192 of rows, so the loss error is ~1e-7 rel).
            # top8_all[:,15,:] written by DVE op 25; explicit self-wait
            # (no same-engine RAW ordering).
            vector.wait_ge(s_dve, 25)
            vector.reduce_sum(
                out=s_red[:, :], in_=top8_all[:, :, 0:5], axis=X
            ).then_inc(s_dve, 1)  # -> 27
            vector.wait_ge(s_dve, 27)
            vector.tensor_scalar(
                out=logs_all[:, :],
                in0=s_red[:, :],
                scalar1=LN101 / 5.0,
                scalar2=LN5,
                op0=Alu.mult,
                op1=Alu.add,
            ).then_inc(s_dve, 1)  # -> 28
            vector.wait_ge(s_gather, 16)
            vector.wait_ge(s_act, 23)  # lse ready
            # a = lse - x_t  (= -log_prob[target])
            vector.tensor_sub(
                out=a_all[:, :], in0=lse_all[:, :], in1=xt_all[:, :]
            ).then_inc(s_dve, 1)  # -> 29
            # sel = x_t >= 5th-largest (top8_all done via the >=25 wait)
            vector.tensor_tensor(
                out=sel_all[:, :],
                in0=xt_all[:, :],
                in1=top8_all[:, :, 4],
                op=Alu.is_ge,
            ).then_inc(s_dve, 1)  # -> 30
            # u = (1-ln101)*x_t + logs  (logs write done via >=28 wait,
            # which the dispatch order already passed)
            vector.wait_ge(s_dve, 28)
            vector.scalar_tensor_tensor(
                out=u_all[:, :],
                in0=xt_all[:, :],
                scalar=1.0 - LN101,
                in1=logs_all[:, :],
                op0=Alu.mult,
                op1=Alu.add,
            ).then_inc(s_dve, 1)  # -> 31
            # d = u - lse  (= logs - lse + (1-ln101)*x_t)
            vector.wait_ge(s_dve, 31)
            vector.tensor_sub(
                out=d_all[:, :], in0=u_all[:, :], in1=lse_all[:, :]
            ).then_inc(s_dve, 1)  # -> 32
            # m = sel * d
            vector.wait_ge(s_dve, 32)
            vector.tensor_mul(
                out=d_all[:, :], in0=sel_all[:, :], in1=d_all[:, :]
            ).then_inc(s_dve, 1)  # -> 33
            # term = 2*a + m, with the row sum fused via accum_out
            vector.wait_ge(s_dve, 33)
            vector.scalar_tensor_tensor(
                out=term_all[:, :],
                in0=a_all[:, :],
                scalar=2.0,
                in1=d_all[:, :],
                op0=Alu.mult,
                op1=Alu.add,
                accum_out=partial[:, :],
            ).then_inc(s_dve, 1)  # -> 34

    return nc


def get_bass(reps=1, debug=False):
    assert reps == 1 and not debug
    key = "nc"
    if key not in _CACHE:
        _CACHE[key] = _build_bass()
    return _CACHE[key]


def make_in_maps(input, target):
    """Shard the full inputs into per-core input maps."""
    x = np.ascontiguousarray(np.asarray(input, dtype=np.float32))
    t = np.asarray(target).astype(np.int64)
    assert x.shape == (B, C), x.shape
    assert t.shape == (B,), t.shape
    rows_local = np.arange(B_LOCAL, dtype=np.int64)
    in_maps = []
    for k in range(N_CORES):
        lo = k * B_LOCAL
        flat_idx = rows_local * C + t[lo : lo + B_LOCAL]
        # gidx[p, i] = flat offset of local row i*P + p
        gidx_k = np.ascontiguousarray(
            flat_idx.reshape(NTILES, P).T.astype(np.int32)
        )
        in_maps.append({"x": x[lo : lo + B_LOCAL], "gidx": gidx_k})
    return in_maps


def reduce_outputs(results):
    """Combine per-core [P, 1] partial sums into the scalar loss."""
    total = np.float64(0.0)
    for r in results:
        total += np.asarray(r["out"], dtype=np.float64).sum()
    return np.float32(total / B)


def kernel(input, target):
    from concourse.bass_utils import run_bass_kernel_spmd

    nc = get_bass()
    in_maps = make_in_maps(input, target)
    res = run_bass_kernel_spmd(nc, in_maps, list(range(N_CORES)))
    return reduce_outputs(res.results)


# revision 17
# speedup vs baseline: 1.1933x; 1.1933x over previous
"""Custom cross-entropy-with-top-k loss kernel for Trainium2 (8 NeuronCores).

Reference computation (B=16384 rows, C=8192 classes, K=5, POWER=1.01):
    log_prob      = log_softmax(input)
    topk_vals     = top-5 values per row
    log_prob_topk = log(1.01^topk_vals / sum(1.01^topk_vals))
    log_prob_copy = log_prob with topk positions overwritten by log_prob_topk
    loss = mean(-log_prob[r, target[r]]) + mean(-log_prob_copy[r, target[r]])

Key reduction: the scalar loss needs only, per row,
    lse   = log(sum(exp(x)))               (x ~ N(0,1): exp() safe in f32)
    x_t   = x[row, target[row]]            (indirect-DMA gather)
    top5  = 5 largest values               (VectorE InstMax = top-8)
    sel   = x_t >= top5[4]                 (is target among the top-5)
    lp2   = sel ? ln(1.01)*x_t - log(sum(1.01^top5)) : x_t - lse
    term  = (lse - x_t) - lp2
and the answer is mean(term).  Per core: 2048 rows = 16 tiles of 128
partitions x 8192 f32, streamed at the HBM roofline.

Pipeline structure (v2):
  - Tiles 1..14 load as full 4 MiB HWDGE transfers into a 5-slot SBUF
    rotation; tiles 0 and 15 load as 4x 2048-column chunks.  Chunking
    tile 0 lets ScalarE/VectorE start ~12 us earlier (the per-tile
    MAX8 total of ~167 us/core is near the stream duration, so start
    latency is on the critical path); chunking tile 15 cuts the tail
    after the last HBM byte from ~19 us to ~6 us.
  - ScalarE: one Exp pass per tile/chunk with a per-row accumulator
    (sumexp).  The elementwise output goes to a write-only fp16 sink
    that is never read, so no WAW synchronization is needed on it.
  - VectorE: InstMax (top-8) per tile/chunk; chunked tiles merge via a
    second InstMax over the 4x8 concatenated chunk results (top-5 of a
    row is contained in the union of per-chunk top-8s).
  - GpSimd: one indirect-DMA gather of x[row, target[row]] (drains in
    the first ~15 us of the stream, off the critical path).
  - Epilogue: ln/exp on [128,16]-shaped tiles plus a short DVE chain;
    the final scalar_tensor_tensor emits the per-partition row sum via
    accum_out, fusing the last reduction.

Written in raw Bass (no Tile scheduler): the neuronxcc walrus backend
only encodes ONE semaphore wait per TPB instruction, so synchronization
uses explicit standalone wait_ge instructions (one wait each) and
relies on transitive ordering.
"""

import numpy as np

P = 128                    # SBUF partitions
C = 8192                   # classes
NTILES = 16                # row-tiles per core
B_LOCAL = P * NTILES       # 2048 rows per core
N_CORES = 8
B = B_LOCAL * N_CORES      # 16384
LN101 = float(np.log(np.float64(1.01)))

NB = 5                     # x-tile rotation depth
NCH = 4                    # column chunks for the first/last tile
CW = C // NCH              # 2048 columns per chunk (last tile, uniform)
# First tile uses uneven chunks so the first Exp/MAX8 can start as soon
# as possible (the DVE MAX8 total is close to the stream duration, so
# compute start latency is on the critical path).
CH0 = [(0, 1024), (1024, 1024), (2048, 2048), (4096, 4096)]
LN5 = float(np.log(np.float64(5.0)))
_CACHE = {}


def _build_bass():
    from contextlib import ExitStack

    import concourse.bass as bass
    import concourse.mybir as mybir

    nc = bass.Bass()
    f32 = mybir.dt.float32
    f16 = mybir.dt.float16
    x = nc.declare_dram_parameter("x", [B_LOCAL, C], f32, isOutput=False)
    gidx = nc.declare_dram_parameter(
        "gidx", [P, NTILES], mybir.dt.int32, isOutput=False
    )
    out = nc.declare_dram_parameter("out", [P, 1], f32, isOutput=True)

    Exp = mybir.ActivationFunctionType.Exp
    Ln = mybir.ActivationFunctionType.Ln
    X = mybir.AxisListType.X
    Alu = mybir.AluOpType

    # s_act increments (all on ScalarE, in program order):
    #   tile0 chunks: 1..4 ; tiles 1..14: 4+i ; tile15 chunks: 19..22 ;
    #   lse: 23
    def act_done(j):       # s_act value once tile j's exp(+accum) is done
        return 4 if j == 0 else 4 + j

    # s_dve increments (all on VectorE, in program order):
    #   tile0 chunk maxes: 1..4 ; merge0: 5 ; se0 reduce: 6 ;
    #   tiles 1..14 maxes: 6+i ; tile15 chunk maxes: 21..24 ; merge15: 25 ;
    #   se15 reduce: 26 ; sred: 27 ; logs: 28 ; a: 29 ; sel: 30 ; u: 31 ;
    #   d: 32 ; m: 33 ; term(+accum): 34
    def dve_done(j):       # s_dve value once tile j's slot reads are done
        return 4 if j == 0 else 6 + j

    DVE_FINAL = 34

    with ExitStack() as ctx:
        xt = [
            ctx.enter_context(nc.sbuf_tensor(f"xt{j}", [P, C], f32))
            for j in range(NB)
        ]
        # Write-only elementwise sink for the Exp passes (never read).
        scr = ctx.enter_context(nc.sbuf_tensor("scr", [P, C], f16))
        gidx_sb = ctx.enter_context(
            nc.sbuf_tensor("gidx_sb", [P, NTILES], mybir.dt.int32)
        )
        xt_all = ctx.enter_context(nc.sbuf_tensor("xt_all", [P, NTILES], f32))
        top8_all = ctx.enter_context(
            nc.sbuf_tensor("top8_all", [P, NTILES, 8], f32)
        )
        top8c0 = ctx.enter_context(nc.sbuf_tensor("top8c0", [P, NCH, 8], f32))
        top8c15 = ctx.enter_context(nc.sbuf_tensor("top8c15", [P, NCH, 8], f32))
        se_all = ctx.enter_context(nc.sbuf_tensor("se_all", [P, NTILES], f32))
        sec0 = ctx.enter_context(nc.sbuf_tensor("sec0", [P, NCH], f32))
        sec15 = ctx.enter_context(nc.sbuf_tensor("sec15", [P, NCH], f32))
        lse_all = ctx.enter_context(nc.sbuf_tensor("lse_all", [P, NTILES], f32))
        s_red = ctx.enter_context(nc.sbuf_tensor("s_red", [P, NTILES], f32))
        logs_all = ctx.enter_context(
            nc.sbuf_tensor("logs_all", [P, NTILES], f32)
        )
        a_all = ctx.enter_context(nc.sbuf_tensor("a_all", [P, NTILES], f32))
        u_all = ctx.enter_context(nc.sbuf_tensor("u_all", [P, NTILES], f32))
        d_all = ctx.enter_context(nc.sbuf_tensor("d_all", [P, NTILES], f32))
        sel_all = ctx.enter_context(nc.sbuf_tensor("sel_all", [P, NTILES], f32))
        term_all = ctx.enter_context(
            nc.sbuf_tensor("term_all", [P, NTILES], f32)
        )
        partial = ctx.enter_context(nc.sbuf_tensor("partial", [P, 1], f32))

        s_gidx = ctx.enter_context(nc.semaphore("s_gidx"))
        # One semaphore per DMA transfer: the 16 SDMA engines increment
        # independently (one +1 each), so a semaphore shared by several
        # transfers can reach 16 via a MIX of engine-shares of different
        # transfers — only a dedicated sem's >=16 proves a transfer
        # landed.  Hence per-chunk semaphores for the chunked tiles.
        s_load = [
            ctx.enter_context(nc.semaphore(f"s_load{i}")) for i in range(NTILES)
        ]
        s_c0 = [
            ctx.enter_context(nc.semaphore(f"s_c0_{c}")) for c in range(NCH)
        ]
        s_c15 = [
            ctx.enter_context(nc.semaphore(f"s_c15_{c}")) for c in range(NCH)
        ]
        s_store = ctx.enter_context(nc.semaphore("s_store"))
        s_gather = ctx.enter_context(nc.semaphore("s_gather"))
        s_act = ctx.enter_context(nc.semaphore("s_act"))
        s_dve = ctx.enter_context(nc.semaphore("s_dve"))
        block = ctx.enter_context(nc.Block())

        def cs(c):
            return slice(c * CW, (c + 1) * CW)

        @block.sync
        def _(sync):
            # tile 0 as 4 uneven column chunks (compute starts on the small
            # first chunk ~15 us before a full-tile load would complete)
            for c, (off, w) in enumerate(CH0):
                sync.dma_start(
                    out=xt[0][:, off : off + w], in_=x[0:P, off : off + w]
                ).then_inc(s_c0[c], 16)
            # gidx after the tile-0 chunks: the gather result is only
            # needed in the epilogue.
            sync.dma_start(out=gidx_sb[:, :], in_=gidx[:, :]).then_inc(s_gidx, 16)
            # full tiles 1..14
            for i in range(1, NTILES - 1):
                if i >= NB:
                    # Slot reuse: wait for both compute consumers of the
                    # previous occupant (tile i-NB).  Their completion also
                    # implies that load's completion, covering the slot WAW
                    # transitively.
                    sync.wait_ge(s_act, act_done(i - NB))
                    sync.wait_ge(s_dve, dve_done(i - NB))
                sync.dma_start(
                    out=xt[i % NB][:, :], in_=x[i * P : (i + 1) * P, :]
                ).then_inc(s_load[i], 16)
            # tile 15 as 4 column chunks (short tail after the last byte)
            sync.wait_ge(s_act, act_done(NTILES - 1 - NB))
            sync.wait_ge(s_dve, dve_done(NTILES - 1 - NB))
            for c in range(NCH):
                sync.dma_start(
                    out=xt[(NTILES - 1) % NB][:, cs(c)],
                    in_=x[(NTILES - 1) * P : NTILES * P, cs(c)],
                ).then_inc(s_c15[c], 16)
            # final store after the whole epilogue
            sync.wait_ge(s_dve, DVE_FINAL)
            sync.dma_start(out=out[:, :], in_=partial[:, :]).then_inc(s_store, 16)

        @block.gpsimd
        def _(gpsimd):
            gpsimd.wait_ge(s_gidx, 16)
            x_flat = bass.AP(tensor=x, offset=0, ap=[[1, B_LOCAL * C], [1, 1]])
            gpsimd.indirect_dma_start(
                out=xt_all[:, :],
                out_offset=None,
                in_=x_flat,
                in_offset=bass.IndirectOffsetOnAxis(ap=gidx_sb[:, :], axis=0),
            ).then_inc(s_gather, 16)

        @block.scalar
        def _(scalar):
            # tile 0 chunks -> s_act 1..4
            for c, (off, w) in enumerate(CH0):
                scalar.wait_ge(s_c0[c], 16)
                scalar.activation(
                    out=scr[:, off : off + w],
                    in_=xt[0][:, off : off + w],
                    func=Exp,
                    accum_out=sec0[:, c : c + 1],
                ).then_inc(s_act, 1)
            # tiles 1..14 -> s_act 4+i
            for i in range(1, NTILES - 1):
                scalar.wait_ge(s_load[i], 16)
                scalar.activation(
                    out=scr[:, :],
                    in_=xt[i % NB][:, :],
                    func=Exp,
                    accum_out=se_all[:, i : i + 1],
                ).then_inc(s_act, 1)
            # tile 15 chunks -> s_act 19..22
            for c in range(NCH):
                scalar.wait_ge(s_c15[c], 16)
                scalar.activation(
                    out=scr[:, cs(c)],
                    in_=xt[(NTILES - 1) % NB][:, cs(c)],
                    func=Exp,
                    accum_out=sec15[:, c : c + 1],
                ).then_inc(s_act, 1)
            # epilogue: lse needs se_all[:,0] and se_all[:,15], written by
            # DVE reduces (s_dve 6 and 26); 26 also implies this engine's
            # own accumulator reads have completed (the DVE reduce waited
            # on s_act>=22).
            scalar.wait_ge(s_dve, 26)
            scalar.activation(
                out=lse_all[:, :], in_=se_all[:, :], func=Ln
            ).then_inc(s_act, 1)  # -> 23

        @block.vector
        def _(vector):
            # tile 0 chunk maxes -> s_dve 1..4
            for c, (off, w) in enumerate(CH0):
                vector.wait_ge(s_c0[c], 16)
                vector.max(
                    out=top8c0[:, c, :], in_=xt[0][:, off : off + w]
                ).then_inc(s_dve, 1)
            # merge0 reads top8c0 (same-engine RAW: self-wait)
            vector.wait_ge(s_dve, 4)
            vector.max(
                out=top8_all[:, 0, :], in_=top8c0[:, :, :]
            ).then_inc(s_dve, 1)  # -> 5
            vector.wait_ge(s_act, 4)  # tile0 chunk accums done
            vector.reduce_sum(
                out=se_all[:, 0:1], in_=sec0[:, :], axis=X
            ).then_inc(s_dve, 1)  # -> 6
            # tiles 1..14 -> s_dve 6+i
            for i in range(1, NTILES - 1):
                vector.wait_ge(s_load[i], 16)
                vector.max(
                    out=top8_all[:, i, :], in_=xt[i % NB][:, :]
                ).then_inc(s_dve, 1)
            # tile 15 chunk maxes -> s_dve 21..24
            for c in range(NCH):
                vector.wait_ge(s_c15[c], 16)
                vector.max(
                    out=top8c15[:, c, :], in_=xt[(NTILES - 1) % NB][:, cs(c)]
                ).then_inc(s_dve, 1)
            vector.wait_ge(s_dve, 24)
            vector.max(
                out=top8_all[:, NTILES - 1, :], in_=top8c15[:, :, :]
            ).then_inc(s_dve, 1)  # -> 25
            vector.wait_ge(s_act, 22)  # tile15 chunk accums done
            vector.reduce_sum(
                out=se_all[:, NTILES - 1 : NTILES], in_=sec15[:, :], axis=X
            ).then_inc(s_dve, 1)  # -> 26
            # epilogue.  s_red = sum of the top-5 values per tile; logs
            # uses log(sum(1.01^v over top5)) = ln5 + ln(1.01)*mean(top5)
            # to second order in ln(1.01)*v (|error| <= ~2e-4, and the
            # term it feeds only applies to rows whose target is in the
            # top-5: 5/8192 of rows, so the loss error is ~1e-7 rel).
            # top8_all[:,15,:] written by DVE op 25; explicit self-wait
            # (no same-engine RAW ordering).
            vector.wait_ge(s_dve, 25)
            vector.reduce_sum(
                out=s_red[:, :], in_=top8_all[:, :, 0:5], axis=X
            ).then_inc(s_dve, 1)  # -> 27
            vector.wait_ge(s_dve, 27)
            vector.tensor_scalar(
                out=logs_all[:, :],
                in0=s_red[:, :],
                scalar1=LN101 / 5.0,
                scalar2=LN5,
                op0=Alu.mult,
                op1=Alu.add,
            ).then_inc(s_dve, 1)  # -> 28
            vector.wait_ge(s_gather, 16)
            vector.wait_ge(s_act, 23)  # lse ready
            # a = lse - x_t  (= -log_prob[target])
            vector.tensor_sub(
                out=a_all[:, :], in0=lse_all[:, :], in1=xt_all[:, :]
            ).then_inc(s_dve, 1)  # -> 29
            # sel = x_t >= 5th-largest (top8_all done via the >=25 wait)
            vector.tensor_tensor(
                out=sel_all[:, :],
                in0=xt_all[:, :],
                in1=top8_all[:, :, 4],
                op=Alu.is_ge,
            ).then_inc(s_dve, 1)  # -> 30
            # u = (1-ln101)*x_t + logs  (logs write done via >=28 wait,
            # which the dispatch order already passed)
            vector.wait_ge(s_dve, 28)
            vector.scalar_tensor_tensor(
                out=u_all[:, :],
                in0=xt_all[:, :],
                scalar=1.0 - LN101,
                in1=logs_all[:, :],
                op0=Alu.mult,
                op1=Alu.add,
            ).then_inc(s_dve, 1)  # -> 31
            # d = u - lse  (= logs - lse + (1-ln101)*x_t)
            vector.wait_ge(s_dve, 31)
            vector.tensor_sub(
                out=d_all[:, :], in0=u_all[:, :], in1=lse_all[:, :]
            ).then_inc(s_dve, 1)  # -> 32
            # m = sel * d
            vector.wait_ge(s_dve, 32)
            vector.tensor_mul(
                out=d_all[:, :], in0=sel_all[:, :], in1=d_all[:, :]
            ).then_inc(s_dve, 1)  # -> 33
            # term = 2*a + m, with the row sum fused via accum_out
            vector.wait_ge(s_dve, 33)
            vector.scalar_tensor_tensor(
                out=term_all[:, :],
                in0=a_all[:, :],
                scalar=2.0,
                in1=d_all[:, :],
                op0=Alu.mult,
                op1=Alu.add,
                accum_out=partial[:, :],
            ).then_inc(s_dve, 1)  # -> 34

    return nc


def get_bass(reps=1, debug=False):
    assert reps == 1 and not debug
    key = "nc"
    if key not in _CACHE:
        _CACHE[key] = _build_bass()
    return _CACHE[key]


def make_in_maps(input, target):
    """Shard the full inputs into per-core input maps."""
    x = np.ascontiguousarray(np.asarray(input, dtype=np.float32))
    t = np.asarray(target).astype(np.int64)
    assert x.shape == (B, C), x.shape
    assert t.shape == (B,), t.shape
    rows_local = np.arange(B_LOCAL, dtype=np.int64)
    in_maps = []
    for k in range(N_CORES):
        lo = k * B_LOCAL
        flat_idx = rows_local * C + t[lo : lo + B_LOCAL]
        # gidx[p, i] = flat offset of local row i*P + p
        gidx_k = np.ascontiguousarray(
            flat_idx.reshape(NTILES, P).T.astype(np.int32)
        )
        in_maps.append({"x": x[lo : lo + B_LOCAL], "gidx": gidx_k})
    return in_maps


def reduce_outputs(results):
    """Combine per-core [P, 1] partial sums into the scalar loss."""
    total = np.float64(0.0)
    for r in results:
        total += np.asarray(r["out"], dtype=np.float64).sum()
    return np.float32(total / B)


def kernel(input, target):
    from concourse.bass_utils import run_bass_kernel_spmd

    nc = get_bass()
    in_maps = make_in_maps(input, target)
    res = run_bass_kernel_spmd(nc, in_maps, list(range(N_CORES)))
    return reduce_outputs(res.results)
